# revision 1
# baseline (speedup 1.0000x reference)
"""Local+global sparse attention (T=4096, D=64, window=512, global stride 64)
for Trainium2, sharded one head per NeuronCore (B*H = 8 = n_cores).

Per-head layout (all hardcoded for T=4096, D=64):
  - 8 query superblocks of 512 queries each.
  - Per superblock s: 8 band k-tiles of 128 keys covering k in
    [512(s-1), 512(s+1)) (only 4 tiles for s=0), masked post-exp: 0/1
    multiplicative masks on E (window edge + stride-64 stripes, DVE) and
    causal triangles via gpsimd affine_select; plus a "global" tile of
    the stride-64 keys k < 512(s-1) (all valid, no mask).
  - S^T layout [k_tile=128 part, q=512 free]: S = K_T.T @ Q_T (f32r).
  - exp on ScalarE (PSUM -> SBUF f32r), no max subtraction (scores are
    O(5) for randn inputs, exp stays finite in fp32).
  - PV: out^T[65, 512] += V_ext[k,65].T-style matmul accumulation where
    V_ext has a ones column producing the softmax denominator Z in row 64.
  - Host divides by Z and transposes back.
"""

import sys

sys.path.insert(0, "/opt/trn_rl_repo")

from contextlib import ExitStack

import numpy as np

import concourse.bass as bass
import concourse.mybir as mybir
import concourse.tile as tile
from concourse import bacc
from concourse.bass_utils import run_bass_kernel_spmd

f32 = mybir.dt.float32
f32r = mybir.dt.float32r
AF = mybir.ActivationFunctionType

T, D = 4096, 64
W, GS = 512, 64
NSB = T // 512            # 8 superblocks
SCALE = 1.0 / 8.0         # 1/sqrt(D)
NEG = -1.0e30

# additive mask pack layout (columns of the [128, 1408] mask tensor):
#   M_low_j0 for j0=0..3 at offsets below, widths 512-128*j0
#   M_diag at offset 1280, width 128
_M_OFF = [0, 512, 896, 1152]
_M_DIAG = 1280
_M_COLS = 1408

TRACE = False
LAST_RESULT = None


def _build_masks():
    kk = np.arange(128)[:, None]
    m = np.zeros((128, _M_COLS), np.float32)
    for j0 in range(4):
        w = 512 - 128 * j0
        r = np.arange(128 * j0, 512)[None, :]
        valid = (r <= kk + 128 * j0) | (kk % GS == 0)
        m[:, _M_OFF[j0] : _M_OFF[j0] + w] = valid.astype(np.float32)
    rr = np.arange(128)[None, :]
    m[:, _M_DIAG : _M_DIAG + 128] = np.where(rr >= kk, 0.0, NEG)
    return m


def _build_nc():
    nc = bacc.Bacc("TRN2", target_bir_lowering=False, debug=False, num_devices=8)
    qt_d = nc.dram_tensor("qt", [64, 4096], f32, kind="ExternalInput")
    kt_d = nc.dram_tensor("kt", [64, 4096], f32, kind="ExternalInput")
    ve_d = nc.dram_tensor("ve", [128, 32 * 65], f32, kind="ExternalInput")
    kg_d = nc.dram_tensor("kg", [64, 64], f32, kind="ExternalInput")
    vg_d = nc.dram_tensor("vg", [128, 65], f32, kind="ExternalInput")
    m_d = nc.dram_tensor("m", [128, _M_COLS], f32, kind="ExternalInput")
    o_d = nc.dram_tensor("o", [NSB, 65, 512], f32, kind="ExternalOutput")

    with tile.TileContext(nc) as tc:
        with ExitStack() as ctx:
            const = ctx.enter_context(tc.tile_pool(name="const", bufs=1))
            ep = ctx.enter_context(tc.tile_pool(name="ep", bufs=6))
            op = ctx.enter_context(tc.tile_pool(name="op", bufs=3))
            ps_s = ctx.enter_context(tc.tile_pool(name="ps_s", bufs=5, space="PSUM"))
            ps_g = ctx.enter_context(tc.tile_pool(name="ps_g", bufs=1, space="PSUM"))
            ps_o = ctx.enter_context(tc.tile_pool(name="ps_o", bufs=2, space="PSUM"))

            # --- load + round inputs to f32r (DVE is the only producer the
            # matmuls ever wait on for SBUF operands) ---
            m_t = const.tile([128, _M_COLS], f32, tag="m_t")
            nc.sync.dma_start(out=m_t[:], in_=m_d[:])

            rtiles = {}
            for name, dh, shape in [
                ("kt", kt_d, [64, 4096]),
                ("qt", qt_d, [64, 4096]),
                ("ve", ve_d, [128, 32 * 65]),
                ("kg", kg_d, [64, 64]),
                ("vg", vg_d, [128, 65]),
            ]:
                stg = const.tile(shape, f32, tag=name + "_s")
                rt = const.tile(shape, f32r, tag=name + "_r")
                half = shape[1] // 2
                if shape[1] >= 2048:
                    nc.sync.dma_start(out=stg[:, :half], in_=dh[:, :half])
                    nc.vector.tensor_copy(rt[:, :half], stg[:, :half])
                    nc.sync.dma_start(out=stg[:, half:], in_=dh[:, half:])
                    nc.vector.tensor_copy(rt[:, half:], stg[:, half:])
                else:
                    nc.sync.dma_start(out=stg[:], in_=dh[:])
                    nc.vector.tensor_copy(rt[:], stg[:])
                rtiles[name] = rt
            qt, kt, ve, kg, vg = (rtiles[k] for k in ["qt", "kt", "ve", "kg", "vg"])

            for s in range(NSB):
                out_ps = ps_o.tile([128, 512], f32, tag="out")
                qcol = 512 * s
                pv_jobs = []

                def band_tile(j0):
                    kti = (4 * (s - 1) + j0) if s >= 1 else (j0 - 4)
                    lower = j0 <= 3
                    sp0 = 0 if lower else 128 * (j0 - 4)
                    sps = ps_s.tile([128, 512], f32, tag="sps")
                    nc.tensor.matmul(
                        sps[:, sp0:512],
                        lhsT=kt[:, 128 * kti : 128 * kti + 128],
                        rhs=qt[:, qcol + sp0 : qcol + 512],
                        start=True,
                        stop=True,
                    )
                    E = ep.tile([128, 512], f32r, tag="E")
                    nc.scalar.activation(E[:, sp0:512], sps[:, sp0:512], AF.Exp, scale=SCALE)
                    if lower:
                        w = 512 - 128 * j0
                        nc.vector.tensor_mul(
                            E[:, 128 * j0 : 512],
                            E[:, 128 * j0 : 512],
                            m_t[:, _M_OFF[j0] : _M_OFF[j0] + w],
                        )
                    if not lower:
                        # zero the non-causal triangle of the boundary block:
                        # keep where (rr - kk) >= 0
                        nc.gpsimd.affine_select(
                            out=E[:, sp0 : sp0 + 128],
                            in_=E[:, sp0 : sp0 + 128],
                            compare_op=mybir.AluOpType.is_ge,
                            fill=0.0,
                            base=0,
                            pattern=[[1, 128]],
                            channel_multiplier=-1,
                        )
                    pv_jobs.append((kti, sp0, E))

                band_tile(4)
                for j0 in ([0, 1, 2, 3] if s >= 1 else []) + [5, 6, 7]:
                    band_tile(j0)

                ng = 8 * (s - 1) if s >= 2 else 0
                Eg = None
                if ng > 0:
                    spg = ps_g.tile([128, 512], f32, tag="spg")
                    nc.tensor.matmul(
                        spg[0:ng, :],
                        lhsT=kg[:, 0:ng],
                        rhs=qt[:, qcol : qcol + 512],
                        start=True,
                        stop=True,
                    )
                    Eg = ep.tile([128, 512], f32r, tag="Eg")
                    nc.scalar.activation(Eg[0:ng, :], spg[0:ng, :], AF.Exp, scale=SCALE)

                n_pv = len(pv_jobs) + (1 if ng > 0 else 0)
                for i, (kti, sp0, E) in enumerate(pv_jobs):
                    nc.tensor.matmul(
                        out_ps[0:65, sp0:512],
                        lhsT=ve[:, 65 * kti : 65 * kti + 65],
                        rhs=E[:, sp0:512],
                        start=(i == 0),
                        stop=(i == n_pv - 1),
                    )
                if ng > 0:
                    nc.tensor.matmul(
                        out_ps[0:65, :],
                        lhsT=vg[0:ng, 0:65],
                        rhs=Eg[0:ng, :],
                        start=False,
                        stop=True,
                    )

                o_sb = op.tile([128, 512], f32, tag="o_sb")
                nc.vector.tensor_copy(o_sb[0:65, :], out_ps[0:65, :])
                nc.sync.dma_start(out=o_d[s], in_=o_sb[0:65, :])

    nc.compile()
    return nc


_CACHE = {}


def _get_nc():
    if "nc" not in _CACHE:
        _CACHE["nc"] = _build_nc()
    return _CACHE["nc"]


def kernel(Q, K, V):
    global LAST_RESULT
    Q = np.ascontiguousarray(np.asarray(Q), dtype=np.float32)
    K = np.ascontiguousarray(np.asarray(K), dtype=np.float32)
    V = np.ascontiguousarray(np.asarray(V), dtype=np.float32)
    B, H, t, d = Q.shape
    assert (B, H, t, d) == (1, 8, T, D)

    masks = _build_masks()

    nc = _get_nc()
    in_maps = []
    for h in range(8):
        q = Q[0, h]
        k = K[0, h]
        v = V[0, h]
        qt2 = np.ascontiguousarray(q.T)      # [64, 4096]
        kt2 = np.ascontiguousarray(k.T)
        ve = np.ones((128, 32 * 65), np.float32)
        vv = v.reshape(32, 128, 64).transpose(1, 0, 2)  # [128, 32, 64]
        ve3 = ve.reshape(128, 32, 65)
        ve3[:, :, :64] = vv
        kg = np.ascontiguousarray(k[::GS, :].T)  # [64, 64] = [d, g]
        vg = np.zeros((128, 65), np.float32)
        vg[:64, :64] = v[::GS, :]
        vg[:64, 64] = 1.0
        in_maps.append(
            dict(qt=qt2, kt=kt2, ve=np.ascontiguousarray(ve), kg=kg, vg=vg,
                 m=masks)
        )

    res = run_bass_kernel_spmd(nc, in_maps, list(range(8)), trace=TRACE)
    LAST_RESULT = res

    out = np.empty((1, 8, T, D), np.float32)
    for h in range(8):
        O = res.results[h]["o"]  # [NSB, 65, 512]
        for s in range(NSB):
            out[0, h, 512 * s : 512 * (s + 1), :] = (O[s, :64, :] / O[s, 64:65, :]).T
    return out



# revision 7
# speedup vs baseline: 1.5578x; 1.5578x over previous
"""Local+global sparse attention (T=4096, D=64, window=512, global stride 64)
for Trainium2, sharded one head per NeuronCore (B*H = 8 = n_cores).

Per-head layout (all hardcoded for T=4096, D=64):
  - 8 query superblocks of 512 queries each.
  - Per superblock s:
      * 4 "upper" k-tiles of 128 keys covering k in [512s, 512(s+1)):
        q range [128i, 512), causal boundary triangle via gpsimd
        affine_select on the first 128 columns of each segment.
      * 4 "lower" k-tiles covering k in [512(s-1), 512s) (s>=1): valid only
        for q offset u <= k offset w (window edge), so matmul/exp/PV are
        restricted to q in [0, 128(i+1)); only the last 128 columns need a
        triangle mask (DVE multiply with a shared [128,128] f16 mask).
        Stride-64 global columns (partitions 0 and 64 of each k-tile) are
        excluded for free with a per-partition -60000 bias on the exp
        activation -- they are covered exactly once by the global tile.
      * A "global" tile of all stride-64 keys k < 512s (ng = 8s <= 56
        partitions), always valid, no mask.
  - S^T layout [k_tile=128 part, q free], all matmul operands fp16
    (1 cycle/row at any size, ~4x the mantissa of bf16). The 4 tiles of a
    group land in one wide PSUM tile so a single fused exp per group runs
    on ScalarE (PSUM -> SBUF f16). Matmul PSUM writes must not cross 2KB
    bank boundaries, so segments are ordered [512,384,128,256] giving
    offsets 0,512,896,1024 -- each inside a bank. No max subtraction
    (scores are O(5) for randn inputs, exp stays finite).
  - Software pipelining: the S matmuls of superblock s+1 are emitted before
    the PV matmuls of superblock s so the (in-order) PE keeps ScalarE fed.
  - PV: out^T[65, q] += V_ext.T @ E where V_ext has a ones column producing
    the softmax denominator Z in row 64. Host divides by Z and transposes.
  - DMA: HWDGE has a fixed ~625ns serialized cost per transfer, so inputs
    are packed into 6 bundled DMAs (the first carries just kt0/qt0 so the
    first matmul starts ~2.5us in) and outputs are paired into 4 DMAs, all
    on the SP queue (gpsimd DMAs would burn ~1us of Pool engine each).
"""

import sys

sys.path.insert(0, "/opt/trn_rl_repo")

from contextlib import ExitStack

import numpy as np

import concourse.bass as bass
import concourse.mybir as mybir
import concourse.tile as tile
from concourse import bacc
from concourse.bass_utils import run_bass_kernel_spmd

f32 = mybir.dt.float32
f16 = mybir.dt.float16
AF = mybir.ActivationFunctionType

T, D = 4096, 64
W, GS = 512, 64
NSB = T // 512            # 8 superblocks
SCALE = 1.0 / 8.0         # 1/sqrt(D)
NEG = -60000.0

# segment offsets inside the fused wide PSUM tiles; every (off, width) pair
# must sit inside one 2KB PSUM bank (512 fp32)
UP_W = [512, 384, 256, 128]
UP_OFF = [0, 512, 1024, 896]          # upper tile i at UP_OFF[i], width UP_W[i]
LO_W = [128, 256, 384, 512]
LO_OFF = [896, 1024, 512, 0]          # lower tile i at LO_OFF[i], width LO_W[i]

# bundle b1 extra-constant column offsets (f16 cols after chunk1's 1284)
B1_ML = 1284          # [128, 128] lower-boundary triangle mask
B1_KG = 1412          # [64, 64] global K^T
B1_VG = 1476          # [128, 65] global V + ones col
B1_BZ = 1541          # [128, 1] stripe-exclusion exp bias
B1_COLS = 1542

TRACE = False
LAST_RESULT = None


def _build_nc():
    nc = bacc.Bacc("TRN2", target_bir_lowering=False, debug=False, num_devices=8)
    # input bundles (f16): b0a = kt0|qt0, b0b = ve0,
    # b1 = chunk1 + consts, b2 = chunks 2,3, b3 = chunks 4,5, b4 = chunks 6,7
    b0a_d = nc.dram_tensor("b0a", [64, 1024], f16, kind="ExternalInput")
    b0b_d = nc.dram_tensor("b0b", [128, 260], f16, kind="ExternalInput")
    b1_d = nc.dram_tensor("b1", [128, B1_COLS], f16, kind="ExternalInput")
    b2_d = nc.dram_tensor("b2", [128, 2568], f16, kind="ExternalInput")
    b3_d = nc.dram_tensor("b3", [128, 2568], f16, kind="ExternalInput")
    b4_d = nc.dram_tensor("b4", [128, 2568], f16, kind="ExternalInput")
    o_d = nc.dram_tensor("o", [4, 65, 1024], f32, kind="ExternalOutput")

    with tile.TileContext(nc) as tc:
        with ExitStack() as ctx:
            const = ctx.enter_context(tc.tile_pool(name="const", bufs=1))
            ep = ctx.enter_context(tc.tile_pool(name="ep", bufs=2))
            op = ctx.enter_context(tc.tile_pool(name="op", bufs=2))
            ps_a = ctx.enter_context(tc.tile_pool(name="ps_a", bufs=1, space="PSUM"))
            ps_c = ctx.enter_context(tc.tile_pool(name="ps_c", bufs=1, space="PSUM"))
            ps_g = ctx.enter_context(tc.tile_pool(name="ps_g", bufs=1, space="PSUM"))
            ps_o = ctx.enter_context(tc.tile_pool(name="ps_o", bufs=1, space="PSUM"))

            b0a = const.tile([64, 1024], f16, tag="b0a")
            b0b = const.tile([128, 260], f16, tag="b0b")
            b1 = const.tile([128, B1_COLS], f16, tag="b1")
            b2 = const.tile([128, 2568], f16, tag="b2")
            b3 = const.tile([128, 2568], f16, tag="b3")
            b4 = const.tile([128, 2568], f16, tag="b4")
            nc.sync.dma_start(out=b0a[:], in_=b0a_d[:])
            nc.sync.dma_start(out=b0b[:], in_=b0b_d[:])
            nc.sync.dma_start(out=b1[:], in_=b1_d[:])
            nc.sync.dma_start(out=b2[:], in_=b2_d[:])
            nc.sync.dma_start(out=b3[:], in_=b3_d[:])
            nc.sync.dma_start(out=b4[:], in_=b4_d[:])

            pair = {2: b2, 3: b2, 4: b3, 5: b3, 6: b4, 7: b4}

            def kt_ap(s):
                if s == 0:
                    return b0a[0:64, 0:512]
                if s == 1:
                    return b1[0:64, 260:772]
                off = 1284 * (s % 2)
                return pair[s][0:64, off + 260 : off + 772]

            def qt_ap(s):
                if s == 0:
                    return b0a[0:64, 512:1024]
                if s == 1:
                    return b1[0:64, 772:1284]
                off = 1284 * (s % 2)
                return pair[s][0:64, off + 772 : off + 1284]

            def ve_ap(s):
                if s == 0:
                    return b0b[:, 0:260]
                if s == 1:
                    return b1[:, 0:260]
                off = 1284 * (s % 2)
                return pair[s][:, off : off + 260]

            ml_t = b1[:, B1_ML : B1_ML + 128]
            kg_t = b1[0:64, B1_KG : B1_KG + 64]
            vg_t = b1[:, B1_VG : B1_VG + 65]
            bz_t = b1[:, B1_BZ : B1_BZ + 1]

            def emit_S(s):
                """S matmuls for superblock s into fresh PSUM tiles."""
                kt_s = kt_ap(s)
                qt_s = qt_ap(s)
                sa = ps_a.tile([128, 1536], f32, tag="sa")
                for i in range(4):
                    sp0 = 128 * i
                    nc.tensor.matmul(
                        sa[:, UP_OFF[i] : UP_OFF[i] + UP_W[i]],
                        lhsT=kt_s[:, 128 * i : 128 * i + 128],
                        rhs=qt_s[:, sp0:512],
                        start=True,
                        stop=True,
                    )
                sc = sg = None
                if s >= 1:
                    kt_p = kt_ap(s - 1)
                    sc = ps_c.tile([128, 1536], f32, tag="sc")
                    for i in range(4):
                        qe = LO_W[i]
                        nc.tensor.matmul(
                            sc[:, LO_OFF[i] : LO_OFF[i] + qe],
                            lhsT=kt_p[:, 128 * i : 128 * i + 128],
                            rhs=qt_s[:, 0:qe],
                            start=True,
                            stop=True,
                        )
                    ng = 8 * s
                    sg = ps_g.tile([128, 512], f32, tag="sg")
                    nc.tensor.matmul(
                        sg[0:ng, :],
                        lhsT=kg_t[:, 0:ng],
                        rhs=qt_s[:],
                        start=True,
                        stop=True,
                    )
                return sa, sc, sg

            cur = emit_S(0)
            o_pair = None
            for s in range(NSB):
                sa, sc, sg = cur
                ng = 8 * s

                # --- fused exps (ACT) ---
                ea = ep.tile([128, 1280], f16, tag="ea")
                nc.scalar.activation(ea[:], sa[:, 0:1280], AF.Exp, scale=SCALE)
                ec = eg = None
                if s >= 1:
                    ec = ep.tile([128, 1280], f16, tag="ec")
                    nc.scalar.activation(
                        ec[:], sc[:, 0:1280], AF.Exp, scale=SCALE, bias=bz_t
                    )
                    eg = ep.tile([128, 512], f16, tag="eg")
                    nc.scalar.activation(eg[0:ng, :], sg[0:ng, :], AF.Exp, scale=SCALE)

                # --- masks ---
                # upper boundary: keep u' - kk >= 0 on first 128 cols of each
                # upper segment (gpsimd affine_select)
                for i in range(4):
                    nc.gpsimd.affine_select(
                        out=ea[:, UP_OFF[i] : UP_OFF[i] + 128],
                        in_=ea[:, UP_OFF[i] : UP_OFF[i] + 128],
                        compare_op=mybir.AluOpType.is_ge,
                        fill=0.0,
                        base=0,
                        pattern=[[1, 128]],
                        channel_multiplier=-1,
                    )
                # lower boundary: keep u - 128i <= kk on last 128 cols of each
                # lower segment (DVE multiply with shared [128,128] mask)
                if s >= 1:
                    for i in range(4):
                        b0 = LO_OFF[i] + LO_W[i] - 128
                        nc.vector.tensor_mul(
                            ec[:, b0 : b0 + 128],
                            ec[:, b0 : b0 + 128],
                            ml_t,
                        )

                # --- software pipeline: S matmuls of s+1 go ahead of PV(s) ---
                cur = emit_S(s + 1) if s + 1 < NSB else None

                # --- PV matmuls (diag first: covers [0:512] with start=True) ---
                out_ps = ps_o.tile([128, 512], f32, tag="out")
                ve_s = ve_ap(s)
                n_pv = 4 + (5 if s >= 1 else 0)
                pv_i = 0
                for i in range(4):
                    sp0 = 128 * i
                    nc.tensor.matmul(
                        out_ps[0:65, sp0:512],
                        lhsT=ve_s[:, 65 * i : 65 * i + 65],
                        rhs=ea[:, UP_OFF[i] : UP_OFF[i] + UP_W[i]],
                        start=(pv_i == 0),
                        stop=(pv_i == n_pv - 1),
                    )
                    pv_i += 1
                if s >= 1:
                    ve_p = ve_ap(s - 1)
                    for i in range(4):
                        qe = LO_W[i]
                        nc.tensor.matmul(
                            out_ps[0:65, 0:qe],
                            lhsT=ve_p[:, 65 * i : 65 * i + 65],
                            rhs=ec[:, LO_OFF[i] : LO_OFF[i] + qe],
                            start=False,
                            stop=False,
                        )
                        pv_i += 1
                    nc.tensor.matmul(
                        out_ps[0:65, :],
                        lhsT=vg_t[0:ng, 0:65],
                        rhs=eg[0:ng, :],
                        start=False,
                        stop=True,
                    )

                # --- output: pair two superblocks per DMA ---
                if s % 2 == 0:
                    o_pair = op.tile([128, 1024], f32, tag="o_pair")
                half = 512 * (s % 2)
                nc.vector.tensor_copy(
                    o_pair[0:65, half : half + 512], out_ps[0:65, :]
                )
                if s % 2 == 1:
                    nc.sync.dma_start(out=o_d[s // 2], in_=o_pair[0:65, :])

    nc.compile()
    return nc


_CACHE = {}


def _get_nc():
    if "nc" not in _CACHE:
        _CACHE["nc"] = _build_nc()
    return _CACHE["nc"]


def _to_f16(x):
    return np.asarray(x, dtype=np.float32).astype(np.float16)


def kernel(Q, K, V):
    global LAST_RESULT
    Q = np.ascontiguousarray(np.asarray(Q), dtype=np.float32)
    K = np.ascontiguousarray(np.asarray(K), dtype=np.float32)
    V = np.ascontiguousarray(np.asarray(V), dtype=np.float32)
    B, H, t, d = Q.shape
    assert (B, H, t, d) == (1, 8, T, D)

    # lower-boundary triangle: keep (column) x <= (partition) kk
    ml = (np.arange(128)[None, :] <= np.arange(128)[:, None]).astype(np.float32)
    bz = np.zeros((128, 1), np.float32)
    bz[0, 0] = NEG
    bz[64, 0] = NEG

    nc = _get_nc()
    in_maps = []
    for h in range(8):
        q = Q[0, h]
        k = K[0, h]
        v = V[0, h]
        qt2 = q.T                              # [64, 4096]
        kt2 = k.T
        ve = np.ones((8, 128, 4 * 65), np.float32)
        vv = v.reshape(8, 4, 128, 64).transpose(0, 2, 1, 3)  # [8, 128, 4, 64]
        ve4 = ve.reshape(8, 128, 4, 65)
        ve4[:, :, :, :64] = vv
        kg = kt2[:, ::GS]                      # [64, 64] = [d, g]
        vg = np.zeros((128, 65), np.float32)
        vg[:64, :64] = v[::GS, :]
        vg[:64, 64] = 1.0

        def chunk(s):
            c = np.zeros((128, 1284), np.float32)
            c[:, 0:260] = ve[s]
            c[0:64, 260:772] = kt2[:, 512 * s : 512 * (s + 1)]
            c[0:64, 772:1284] = qt2[:, 512 * s : 512 * (s + 1)]
            return c

        b0a = np.concatenate(
            [kt2[:, 0:512], qt2[:, 0:512]], axis=1
        )                                      # [64, 1024]
        b0b = ve[0]
        b1 = np.zeros((128, B1_COLS), np.float32)
        b1[:, 0:1284] = chunk(1)
        b1[:, B1_ML : B1_ML + 128] = ml
        b1[0:64, B1_KG : B1_KG + 64] = kg
        b1[:, B1_VG : B1_VG + 65] = vg
        b1[:, B1_BZ] = bz[:, 0]
        b2 = np.concatenate([chunk(2), chunk(3)], axis=1)
        b3 = np.concatenate([chunk(4), chunk(5)], axis=1)
        b4 = np.concatenate([chunk(6), chunk(7)], axis=1)
        in_maps.append(
            dict(
                b0a=_to_f16(b0a),
                b0b=_to_f16(b0b),
                b1=_to_f16(b1),
                b2=_to_f16(b2),
                b3=_to_f16(b3),
                b4=_to_f16(b4),
            )
        )

    res = run_bass_kernel_spmd(nc, in_maps, list(range(8)), trace=TRACE)
    LAST_RESULT = res

    out = np.empty((1, 8, T, D), np.float32)
    for h in range(8):
        O = res.results[h]["o"]  # [4, 65, 1024]
        for s in range(NSB):
            blk = O[s // 2][:, 512 * (s % 2) : 512 * (s % 2) + 512]
            out[0, h, 512 * s : 512 * (s + 1), :] = (blk[:64, :] / blk[64:65, :]).T
    return out
